# revision 42
# baseline (speedup 1.0000x reference)
"""GCN edge-aggregation kernel for 8 Trainium2 NeuronCores.

Math (see nn_GCNEdge): h = relu((segment_sum(edge_data, dst) / max(count,1)) @ W.T + b)

Strategy
--------
Host-side (sharding/layout only — all arithmetic happens on device):
  * Nodes live in 1568 sub-blocks of 64; each edge is routed to the sub-block
    owning its destination node (CSR-style destination binning).  Sub-blocks
    are bin-packed onto the 8 cores: sorted by chunk count (ceil(edges/128))
    and dealt round-robin, so every core sees the same per-position
    chunk-count sequence kb_seq and one SPMD program serves all cores.
    64-node sub-blocks (vs 128) halve the DVE one-hot work — the dominant
    on-device cost — at ~3% extra slot padding.
  * Edge features ship as fp8 e3m4 (rel-err gate is 2e-2; e3m4 end-to-end is
    ~1.5e-2), with a constant-1 count column riding along for the degree counts.
  * The xe stream is partition-major: each SBUF partition's data for a run of
    sub-blocks is one contiguous HBM range, so multi-block DMAs move ~13KB per
    partition per transfer (big descriptors -> full HBM bandwidth).

Device-side (per core, per 64-node sub-block):
  * One-hot matrix of local node ids (DVE is_equal against an iota pattern).
    The one-hot is laid out [partition=edge, (node f, chunk c)] — f-major with
    the chunk axis innermost — so every DVE operand has a unit-stride last
    axis, which qualifies the op for the DVE 2x_1p fast path (2 elem/cycle).
    The PE matmul then reads each chunk's one-hot with a stride-K node axis.
  * PE matmul-accumulate onehot.T @ [x | 1] into PSUM -> per-node feature
    sums and counts.  Two sub-blocks (a pair) share one [128, 129] PSUM tile,
    the even one accumulating into partitions 0-63 and the odd one into
    64-127, so the epilogue below runs once per 128 nodes as before.
  * mean = sums * reciprocal(count) (ACT copy with per-partition scale,
    casting to bf16),
  * PE transpose (bf16), then out = relu(W @ agg.T + b) via a bf16 matmul with
    the (pre-transposed) weight as the stationary operand; output stays
    transposed [out_feat, node] in bf16 and is un-transposed / upcast on host.

No collectives are needed: output shards are disjoint.
"""

import numpy as np
import ml_dtypes

BF16 = ml_dtypes.bfloat16

N_NODES = 100000
N_EDGES = 1600000
F = 128
N_CORES = 8
SBLK = 64                       # nodes per sub-block
SUBS_PER_CORE = 196
TOTAL_SUBS = N_CORES * SUBS_PER_CORE            # 1568 >= ceil(100000/64)
NODES_PER_CORE = SUBS_PER_CORE * SBLK           # 12544
XCOL = 129                      # 128 features + count col

_module_cache = {}


def _make_groups():
    """Positions per xe DMA transfer: tapered head so compute starts early,
    big groups in the middle for bandwidth, tapered tail so the final blocks'
    compute overlaps the last transfers."""
    return [4, 4, 6, 8, 10] + [12] * 13 + [4, 2, 1, 1]


def _build_module(kb_seq):
    import concourse.mybir as mybir
    import concourse.tile as tile
    from concourse import bacc

    f32 = mybir.dt.float32
    bf16 = mybir.dt.bfloat16
    fp8 = mybir.dt.float8e3

    kb_seq = list(kb_seq)
    nsubs = len(kb_seq)
    npairs = nsubs // 2
    CH = sum(kb_seq)                       # total chunks per core
    prefix = np.concatenate([[0], np.cumsum(kb_seq)]).astype(int)
    kdistinct = sorted(set(kb_seq))
    ioff = {}
    o = 0
    for k in kdistinct:
        ioff[k] = o
        o += k * SBLK
    IOTA_COLS = o
    KMAX = max(kb_seq)

    groups = _make_groups()
    assert sum(groups) == nsubs
    gstart = np.concatenate([[0], np.cumsum(groups)]).astype(int)
    GMAXCH = max(
        prefix[gstart[gi + 1]] - prefix[gstart[gi]] for gi in range(len(groups))
    )

    # Greedy byte-balance the xe groups over the two hardware DMA queues
    # (SP, ACT); ACT is pre-loaded with the lid constant and the out writes.
    gbytes = [
        float(prefix[gstart[gi + 1]] - prefix[gstart[gi]]) for gi in range(len(groups))
    ]
    act_extra = 2 * (CH + nsubs * SBLK) / XCOL
    load = {0: 0.0, 1: act_extra}
    gqueue = []
    for w in gbytes:
        qsel = 0 if load[0] <= load[1] else 1
        gqueue.append(qsel)
        load[qsel] += w

    nc = bacc.Bacc("TRN2", target_bir_lowering=False, debug=False)
    xe = nc.dram_tensor("xe", [128, CH * XCOL], fp8, kind="ExternalInput")
    lid = nc.dram_tensor("lid", [128, CH], bf16, kind="ExternalInput")
    wt = nc.dram_tensor("wt", [128, 128], bf16, kind="ExternalInput")
    bias = nc.dram_tensor("bias", [128, 1], f32, kind="ExternalInput")
    ident = nc.dram_tensor("ident", [128, 128], bf16, kind="ExternalInput")
    out = nc.dram_tensor("out", [128, nsubs * SBLK], bf16, kind="ExternalOutput")

    xe_ap = xe.ap()
    out_ap = out.ap()

    with tile.TileContext(nc) as tc:
        with (
            tc.tile_pool(name="const", bufs=1) as cpool,
            tc.tile_pool(name="xp", bufs=5) as xpool,
            tc.tile_pool(name="ohp", bufs=12) as ohpool,
            tc.tile_pool(name="ep", bufs=3) as epool,
            tc.tile_pool(name="psS", bufs=4, space="PSUM") as psS,
            tc.tile_pool(name="psT", bufs=2, space="PSUM") as psT,
            tc.tile_pool(name="psO", bufs=2, space="PSUM") as psO,
        ):
            # Constants ride the ACT engine's hardware DMA queue so the SP
            # queue starts the bulk xe stream immediately; the one-hot
            # prerequisite (lid) loads first.  The iota compare patterns
            # (iotafc[p, ioff[k] + f*k + c] = f, one per distinct kb; values
            # 0..63 are exact in bf16) are generated on the idle GPSIMD
            # engine instead of being shipped over HBM — the pattern used by
            # the first blocks is generated first.
            lid_t = cpool.tile([128, CH], bf16)
            nc.scalar.dma_start(lid_t[:], lid.ap()[:])
            wt_t = cpool.tile([128, 128], bf16)
            nc.scalar.dma_start(wt_t[:], wt.ap()[:])
            bias_t = cpool.tile([128, 1], f32)
            nc.scalar.dma_start(bias_t[:], bias.ap()[:])
            id_t = cpool.tile([128, 128], bf16)
            nc.scalar.dma_start(id_t[:], ident.ap()[:])
            iotafc_t = cpool.tile([128, IOTA_COLS], bf16)
            for k in sorted(kdistinct, reverse=True):
                nc.gpsimd.iota(
                    iotafc_t[:, ioff[k]:ioff[k] + k * SBLK],
                    [[1, SBLK], [0, k]],
                    channel_multiplier=0,
                    allow_small_or_imprecise_dtypes=True,
                )

            group_pT = {}
            pair_ps = {}

            def emit_matmuls(j, xt, oh):
                kb = kb_seq[j]
                half = j & 1
                if half == 0:
                    pair_ps["t"] = psS.tile(
                        [128, XCOL], f32, name=f"ps{j}", tag="ps"
                    )
                ps = pair_ps["t"]
                ohv = oh[:, 0:kb * SBLK].rearrange("p (f c) -> p c f", c=kb)
                for c in range(kb):
                    nc.tensor.matmul(
                        ps[half * SBLK:(half + 1) * SBLK, :],
                        lhsT=ohv[:, c, :],
                        rhs=xt[:, c * XCOL:(c + 1) * XCOL],
                        start=(c == 0),
                        stop=(c == kb - 1),
                    )
                return ps

            def emit_scale(pr, ps):
                # counts live in ps[:,128].  No max(count,1) guard: the host
                # guarantees every real node has count > 0 (injecting
                # unit-weight zero-feature phantom edges if needed); padding
                # nodes divide by zero -> NaN columns that the host slices off.
                rec = epool.tile([128, 1], f32, name=f"rec{pr}", tag="rec")
                nc.vector.reciprocal(rec[:], ps[:, 128:129])
                agg = epool.tile([128, 128], bf16, name=f"agg{pr}", tag="agg", bufs=5)
                nc.scalar.activation(
                    agg[:], ps[:, 0:128],
                    mybir.ActivationFunctionType.Copy, scale=rec[:, 0:1],
                )
                return agg

            def emit_tail(pr, agg):
                j = pr % 4
                if j == 0:
                    group_pT["t"] = psT.tile([128, 512], bf16, name=f"pT{pr}", tag="pT")
                pT = group_pT["t"]
                nc.tensor.transpose(pT[:, j * 128:(j + 1) * 128], agg[:], id_t[:])
                if j == 3 or pr == npairs - 1:
                    g0 = (pr // 4) * 4
                    gw = (pr + 1 - g0) * 128
                    aggT = epool.tile([128, 512], bf16, name=f"aggT{pr}", tag="aggT", bufs=3)
                    nc.scalar.copy(aggT[:, 0:gw], pT[:, 0:gw])
                    pO = psO.tile([128, 512], f32, name=f"pO{pr}", tag="pO")
                    nc.tensor.matmul(
                        pO[:, 0:gw], lhsT=wt_t[:], rhs=aggT[:, 0:gw],
                        start=True, stop=True,
                    )
                    ot = epool.tile([128, 512], bf16, name=f"ot{pr}", tag="ot", bufs=4)
                    nc.scalar.activation(
                        ot[:, 0:gw], pO[:, 0:gw],
                        mybir.ActivationFunctionType.Relu,
                        bias=bias_t[:, 0:1], scale=1.0,
                    )
                    nc.scalar.dma_start(out_ap[:, g0 * 128:(pr + 1) * 128], ot[:, 0:gw])

            # One-hot builds are emitted a few positions ahead of their
            # consumer so PE never waits on a freshly-issued DVE op.
            OH_LEAD = 6
            emit_at = {}
            for j in range(nsubs):
                emit_at.setdefault(max(0, j - OH_LEAD), []).append(j)

            # Software-pipelined emission. Every engine queue is strict
            # in-order, so an op gated on *fresh* upstream state stalls the
            # whole queue behind it. Stagger each stage so, by the time a
            # queue reaches an op, its dependencies are positions old:
            #   iter j:  DMA xe group | one-hot(j+6) | PE matmuls(j-1)
            #            | reciprocal+scale of pair (j-5)/2 | tail of (j-9)/2
            xt_of = {}
            gi = 0
            pending = {}
            pending_ps = {}
            pending_agg = {}
            for j in range(nsubs):
                if gi < len(groups) and j == gstart[gi]:
                    c0, c1 = prefix[gstart[gi]], prefix[gstart[gi + 1]]
                    xg = xpool.tile([128, GMAXCH * XCOL], fp8, name=f"xg{gi}", tag="xg")
                    # Both hardware DMA queues (SP and ACT) pull HBM
                    # concurrently, byte-balanced.
                    dma_eng = nc.sync if gqueue[gi] == 0 else nc.scalar
                    dma_eng.dma_start(
                        xg[:, 0:(c1 - c0) * XCOL],
                        xe_ap[:, c0 * XCOL:c1 * XCOL],
                    )
                    for bb in range(gstart[gi], gstart[gi + 1]):
                        off = (prefix[bb] - c0) * XCOL
                        xt_of[bb] = xg[:, off:off + kb_seq[bb] * XCOL]
                    gi += 1
                for jj in emit_at.get(j, ()):
                    kb = kb_seq[jj]
                    oh = ohpool.tile([128, KMAX * SBLK], bf16, name=f"oh{jj}", tag="oh")
                    nc.vector.tensor_tensor(
                        out=oh[:, 0:kb * SBLK]
                            .rearrange("p (f c) -> p f c", c=kb),
                        in0=iotafc_t[:, ioff[kb]:ioff[kb] + kb * SBLK]
                            .rearrange("p (f c) -> p f c", c=kb),
                        in1=lid_t[:, prefix[jj]:prefix[jj] + kb]
                            .rearrange("p (o c) -> p o c", o=1)
                            .to_broadcast([128, SBLK, kb]),
                        op=mybir.AluOpType.is_equal,
                    )
                    pending[jj] = oh
                if j >= 1:
                    bb = j - 1
                    ps = emit_matmuls(bb, xt_of.pop(bb), pending.pop(bb))
                    if bb & 1:
                        pending_ps[bb >> 1] = ps
                if j >= 5 and (j - 5) & 1:
                    pr = (j - 5) >> 1
                    pending_agg[pr] = emit_scale(pr, pending_ps.pop(pr))
                if j >= 9 and (j - 9) & 1:
                    emit_tail((j - 9) >> 1, pending_agg.pop((j - 9) >> 1))
            last = nsubs - 1
            ps = emit_matmuls(last, xt_of.pop(last), pending.pop(last))
            pending_ps[last >> 1] = ps
            for pr in sorted(pending_ps):
                pending_agg[pr] = emit_scale(pr, pending_ps.pop(pr))
            for pr in sorted(pending_agg):
                emit_tail(pr, pending_agg.pop(pr))

    nc.compile()
    return nc


def _get_module(kb_seq):
    key = tuple(kb_seq)
    if key not in _module_cache:
        _module_cache[key] = _build_module(key)
    return _module_cache[key]


def prepare_inputs(edge_data, dst, W, b):
    """Host-side sharding: route each edge to the core/sub-block owning dst."""
    edge_data = np.asarray(edge_data, dtype=np.float32)
    dst = np.asarray(dst)
    W = np.asarray(W, dtype=np.float32)
    b = np.asarray(b, dtype=np.float32)
    E = dst.shape[0]

    # The device kernel divides by the raw count (no max(count,1) guard).
    # Give any zero-degree real node a phantom edge with zero features and a
    # unit count weight: sums stay exactly 0, so mean = 0/1 = 0, which
    # matches the reference's 0/max(0,1).
    node_cnt = np.bincount(dst, minlength=N_NODES)[:N_NODES]
    zeros = np.nonzero(node_cnt == 0)[0]
    n_real = E
    if len(zeros):
        dst = np.concatenate([dst, zeros.astype(dst.dtype)])
        E = dst.shape[0]

    blk = (dst.astype(np.int64)) >> 6                 # destination sub-block
    cnt = np.bincount(blk, minlength=TOTAL_SUBS)
    kb_all = np.maximum(1, -(-cnt // 128))            # chunks per sub-block

    # Bin-pack: sort sub-blocks by chunk count desc, deal round-robin to
    # cores.  Every core then has the same chunk-count sequence kb_seq
    # (per-position max over cores = the first core's, since the deal
    # preserves order).
    sortidx = np.argsort(-kb_all, kind="stable")
    core_of = np.empty(TOTAL_SUBS, np.int64)
    pos_of = np.empty(TOTAL_SUBS, np.int64)
    r = np.arange(TOTAL_SUBS)
    core_of[sortidx] = r % N_CORES
    pos_of[sortidx] = r // N_CORES
    kb_seq = kb_all[sortidx[0::N_CORES]]
    CH = int(kb_seq.sum())
    prefix = np.concatenate([[0], np.cumsum(kb_seq)]).astype(np.int64)

    starts = np.zeros(TOTAL_SUBS, np.int64)
    np.cumsum(cnt[:-1], out=starts[1:])
    order = np.argsort(blk, kind="stable")
    rank = np.empty(E, np.int64)
    rank[order] = np.arange(E, dtype=np.int64) - np.repeat(starts, cnt)

    # Flat slot in the per-core partition-major layout:
    #   (core*128 + partition) * CH + prefix[pos] + chunk
    slot = (
        (core_of[blk] * 128 + (rank & 127)) * CH
        + prefix[pos_of[blk]] + (rank >> 7)
    )

    FP8 = ml_dtypes.float8_e3m4
    X = np.zeros((N_CORES * 128 * CH, XCOL), FP8)
    X[slot[:n_real], 0:128] = edge_data.astype(FP8)
    X[slot[:n_real], 128] = FP8(1.0)
    if len(zeros):
        X[slot[n_real:], 128] = FP8(1.0)
    X = X.reshape(N_CORES, 128, CH * XCOL)

    lid_f = np.full(N_CORES * 128 * CH, -1.0, np.float32)
    lid_f[slot] = (dst & (SBLK - 1)).astype(np.float32)
    lid_all = lid_f.reshape(N_CORES, 128, CH).astype(BF16)

    wt = np.ascontiguousarray(W.T).astype(BF16)
    bias = np.ascontiguousarray(b.reshape(128, 1))
    ident = np.eye(128, dtype=np.float32).astype(BF16)

    in_maps = [
        {
            "xe": np.ascontiguousarray(X[c]),
            "lid": np.ascontiguousarray(lid_all[c]),
            "wt": wt,
            "bias": bias,
            "ident": ident,
        }
        for c in range(N_CORES)
    ]
    return kb_seq, sortidx, in_maps


def run(edge_data, dst, W, b, trace=False, tmpdir=None):
    from concourse.bass_utils import run_bass_kernel_spmd

    kb_seq, sortidx, in_maps = prepare_inputs(edge_data, dst, W, b)
    nc = _get_module(kb_seq)
    res = run_bass_kernel_spmd(
        nc, in_maps, core_ids=list(range(N_CORES)), trace=trace, tmpdir=tmpdir,
    )
    full = np.empty((TOTAL_SUBS * SBLK, 128), np.float32)
    for c in range(N_CORES):
        oc = res.results[c]["out"].T.astype(np.float32)   # [12544, 128]
        subs = sortidx[c::N_CORES]                        # sub-block at pos j
        for j, sb in enumerate(subs):
            full[sb * SBLK:(sb + 1) * SBLK] = oc[j * SBLK:(j + 1) * SBLK]
    full = full[:N_NODES]
    return np.ascontiguousarray(full, dtype=np.float32), res


def kernel(edge_data, dst, W, b):
    out, _ = run(edge_data, dst, W, b, trace=False)
    return out


# revision 49
# speedup vs baseline: 1.3719x; 1.3719x over previous
"""GCN edge-aggregation kernel for 8 Trainium2 NeuronCores.

Math (see nn_GCNEdge): h = relu((segment_sum(edge_data, dst) / max(count,1)) @ W.T + b)

Strategy
--------
Host-side (sharding/layout only — all arithmetic happens on device):
  * Nodes live in 784 blocks of 128; each edge is routed to the block owning
    its destination node (CSR-style destination binning).  Blocks are
    bin-packed onto the 8 cores: sorted by chunk count (ceil(edges/128)) and
    dealt round-robin, so every core sees the same per-position chunk-count
    sequence kb_seq and one SPMD program serves all cores, with only ~4% slot
    padding (vs ~13% for a uniform 18-chunk capacity).
  * Edge features ship as fp8 e3m4 (rel-err gate is 2e-2; e3m4 end-to-end is
    ~1.5e-2), with a constant-1 count column riding along for the degree counts.
  * The xe stream is partition-major: each SBUF partition's data for a run of
    blocks is one contiguous HBM range, so multi-block DMAs move ~30KB per
    partition per transfer (big descriptors -> full HBM bandwidth).

Device-side (per core, per 128-node block):
  * One-hot matrix of local node ids (DVE is_equal against an iota pattern).
    The one-hot is laid out [partition=edge, (node f, chunk c)] — f-major with
    the chunk axis innermost — so every DVE operand has a unit-stride last
    axis, which qualifies the op for the DVE 2x_1p fast path (2 elem/cycle).
    The PE matmul then reads each chunk's one-hot with a stride-K node axis.
  * PE matmul-accumulate onehot.T @ [x | 1 | 0] into PSUM -> per-node feature
    sums and counts,
  * mean = sums * reciprocal(count) (ACT copy with per-partition scale,
    casting to bf16),
  * PE transpose (bf16), then out = relu(W @ agg.T + b) via a bf16 matmul with
    the (pre-transposed) weight as the stationary operand; output stays
    transposed [out_feat, node] in bf16 and is un-transposed / upcast on host.

No collectives are needed: output shards are disjoint.
"""

import numpy as np
import ml_dtypes

BF16 = ml_dtypes.bfloat16

N_NODES = 100000
N_EDGES = 1600000
F = 128
N_CORES = 8
BLK = 128                       # nodes per block
BLOCKS_PER_CORE = 98
TOTAL_BLOCKS = N_CORES * BLOCKS_PER_CORE        # 784
NODES_PER_CORE = BLOCKS_PER_CORE * BLK          # 12544
XCOL = 129                      # 128 features + count col

_module_cache = {}


def _make_groups():
    """Positions per xe DMA transfer: tapered head so compute starts early,
    big groups in the middle for bandwidth, tapered tail so the final blocks'
    compute overlaps the last transfers."""
    return [2, 2, 3, 4, 5] + [6] * 13 + [2, 1, 1]


def _build_module(kb_seq):
    import concourse.mybir as mybir
    import concourse.tile as tile
    from concourse import bacc

    f32 = mybir.dt.float32
    bf16 = mybir.dt.bfloat16
    fp8 = mybir.dt.float8e3

    kb_seq = list(kb_seq)
    nblocks = len(kb_seq)
    CH = sum(kb_seq)                       # total chunks per core
    prefix = np.concatenate([[0], np.cumsum(kb_seq)]).astype(int)
    kdistinct = sorted(set(kb_seq))
    ioff = {}
    o = 0
    for k in kdistinct:
        ioff[k] = o
        o += k * 128
    IOTA_COLS = o
    KMAX = max(kb_seq)

    groups = _make_groups()
    assert sum(groups) == nblocks
    gstart = np.concatenate([[0], np.cumsum(groups)]).astype(int)
    GMAXCH = max(
        prefix[gstart[gi + 1]] - prefix[gstart[gi]] for gi in range(len(groups))
    )

    # Greedy byte-balance the xe groups over the two hardware DMA queues
    # (SP, ACT); ACT is pre-loaded with the lid constant and the out writes.
    gbytes = [
        float(prefix[gstart[gi + 1]] - prefix[gstart[gi]]) for gi in range(len(groups))
    ]
    act_extra = 2 * (nblocks * 128) / XCOL  # out writes (2B/elem), in xe-chunk units
    sp_extra = 2 * CH / XCOL                # lid rides the SP queue
    load = {0: sp_extra, 1: act_extra}
    gqueue = []
    for w in gbytes:
        qsel = 0 if load[0] <= load[1] else 1
        gqueue.append(qsel)
        load[qsel] += w

    nc = bacc.Bacc("TRN2", target_bir_lowering=False, debug=False)
    xe = nc.dram_tensor("xe", [128, CH * XCOL], fp8, kind="ExternalInput")
    lid = nc.dram_tensor("lid", [128, CH], bf16, kind="ExternalInput")
    wt = nc.dram_tensor("wt", [128, 128], bf16, kind="ExternalInput")
    bias = nc.dram_tensor("bias", [128, 1], f32, kind="ExternalInput")
    ident = nc.dram_tensor("ident", [128, 128], bf16, kind="ExternalInput")
    out = nc.dram_tensor("out", [128, nblocks * 128], bf16, kind="ExternalOutput")

    xe_ap = xe.ap()
    out_ap = out.ap()

    with tile.TileContext(nc) as tc:
        with (
            tc.tile_pool(name="const", bufs=1) as cpool,
            tc.tile_pool(name="xp", bufs=5) as xpool,
            tc.tile_pool(name="ohp", bufs=8) as ohpool,
            tc.tile_pool(name="ep", bufs=3) as epool,
            tc.tile_pool(name="psS", bufs=4, space="PSUM") as psS,
            tc.tile_pool(name="psT", bufs=2, space="PSUM") as psT,
            tc.tile_pool(name="psO", bufs=2, space="PSUM") as psO,
        ):
            # Constants ride the ACT engine's hardware DMA queue so the SP
            # queue starts the bulk xe stream immediately; the one-hot
            # prerequisite (lid) loads first.  The iota compare patterns
            # (iotafc[p, ioff[k] + f*k + c] = f, one per distinct kb; values
            # 0..127 are exact in bf16) are generated on the idle GPSIMD
            # engine instead of being shipped over HBM — the pattern used by
            # the first blocks is generated first.
            lid_t = cpool.tile([128, CH], bf16)
            nc.sync.dma_start(lid_t[:], lid.ap()[:])
            wt_t = cpool.tile([128, 128], bf16)
            nc.scalar.dma_start(wt_t[:], wt.ap()[:])
            bias_t = cpool.tile([128, 1], f32)
            nc.scalar.dma_start(bias_t[:], bias.ap()[:])
            id_t = cpool.tile([128, 128], bf16)
            nc.scalar.dma_start(id_t[:], ident.ap()[:])
            iotafc_t = cpool.tile([128, IOTA_COLS], bf16)
            for k in sorted(kdistinct, reverse=True):
                nc.gpsimd.iota(
                    iotafc_t[:, ioff[k]:ioff[k] + k * 128],
                    [[1, 128], [0, k]],
                    channel_multiplier=0,
                    allow_small_or_imprecise_dtypes=True,
                )

            group_pT = {}

            def emit_matmuls(b, xt, oh):
                kb = kb_seq[b]
                ps = psS.tile([128, XCOL], f32, name=f"ps{b}", tag="ps")
                ohv = oh[:, 0:kb * 128].rearrange("p (f c) -> p c f", c=kb)
                for c in range(kb):
                    nc.tensor.matmul(
                        ps[:],
                        lhsT=ohv[:, c, :],
                        rhs=xt[:, c * XCOL:(c + 1) * XCOL],
                        start=(c == 0),
                        stop=(c == kb - 1),
                    )
                return ps

            def emit_scale(b, ps):
                # counts live in ps[:,128].  No max(count,1) guard: the host
                # guarantees every real node has count > 0 (injecting
                # zero-feature phantom edges if needed); padding nodes divide
                # by zero -> NaN columns that the host slices off.
                rec = epool.tile([128, 1], f32, name=f"rec{b}", tag="rec")
                nc.vector.reciprocal(rec[:], ps[:, 128:129])
                agg = epool.tile([128, 128], bf16, name=f"agg{b}", tag="agg", bufs=5)
                nc.scalar.activation(
                    agg[:], ps[:, 0:128],
                    mybir.ActivationFunctionType.Copy, scale=rec[:, 0:1],
                )
                return agg

            def emit_tail(b, agg):
                j = b % 4
                if j == 0:
                    group_pT["t"] = psT.tile([128, 512], bf16, name=f"pT{b}", tag="pT")
                pT = group_pT["t"]
                nc.tensor.transpose(pT[:, j * 128:(j + 1) * 128], agg[:], id_t[:])
                if j == 3 or b == nblocks - 1:
                    g0 = (b // 4) * 4
                    gw = (b + 1 - g0) * 128
                    aggT = epool.tile([128, 512], bf16, name=f"aggT{b}", tag="aggT", bufs=3)
                    nc.scalar.copy(aggT[:, 0:gw], pT[:, 0:gw])
                    pO = psO.tile([128, 512], f32, name=f"pO{b}", tag="pO")
                    nc.tensor.matmul(
                        pO[:, 0:gw], lhsT=wt_t[:], rhs=aggT[:, 0:gw],
                        start=True, stop=True,
                    )
                    ot = epool.tile([128, 512], bf16, name=f"ot{b}", tag="ot", bufs=4)
                    nc.scalar.activation(
                        ot[:, 0:gw], pO[:, 0:gw],
                        mybir.ActivationFunctionType.Relu,
                        bias=bias_t[:, 0:1], scale=1.0,
                    )
                    nc.scalar.dma_start(out_ap[:, g0 * 128:(b + 1) * 128], ot[:, 0:gw])

            # One is_equal builds the one-hots for a whole run of same-kb
            # blocks (up to OH_BATCH), amortizing the per-op fixed cost on
            # the DVE (the busiest engine).
            OH_BATCH = 1
            batches = []              # (b0, R)
            b0 = 0
            while b0 < nblocks:
                R = 1
                while (
                    R < OH_BATCH
                    and b0 + R < nblocks
                    and kb_seq[b0 + R] == kb_seq[b0]
                ):
                    R += 1
                batches.append((b0, R))
                b0 += R
            batch_at = {b0: (b0, R) for b0, R in batches}
            OH_LEAD = 3
            emit_at = {}
            for b0, R in batches:
                emit_at.setdefault(max(0, b0 - OH_LEAD), []).append(b0)

            # Software-pipelined emission. Every engine queue is strict
            # in-order, so an op gated on *fresh* upstream state stalls the
            # whole queue behind it. Stagger each stage so, by the time a
            # queue reaches an op, its dependencies are blocks old:
            #   iter b:  DMA xe group | one-hot batch | PE matmuls(b-1)
            #            | reciprocal+scale of (b-3) | transpose/output of (b-5)
            xt_of = {}
            gi = 0
            pending = {}
            pending_ps = {}
            pending_agg = {}
            for b in range(nblocks):
                if gi < len(groups) and b == gstart[gi]:
                    c0, c1 = prefix[gstart[gi]], prefix[gstart[gi + 1]]
                    xg = xpool.tile([128, GMAXCH * XCOL], fp8, name=f"xg{gi}", tag="xg")
                    # Both hardware DMA queues (SP and ACT) pull HBM
                    # concurrently, byte-balanced.
                    dma_eng = nc.sync if gqueue[gi] == 0 else nc.scalar
                    dma_eng.dma_start(
                        xg[:, 0:(c1 - c0) * XCOL],
                        xe_ap[:, c0 * XCOL:c1 * XCOL],
                    )
                    for bb in range(gstart[gi], gstart[gi + 1]):
                        off = (prefix[bb] - c0) * XCOL
                        xt_of[bb] = xg[:, off:off + kb_seq[bb] * XCOL]
                    gi += 1
                # Emit one-hot batches a few iterations ahead of their first
                # consumer so PE never waits on a freshly-issued DVE op.
                for bb0 in emit_at.get(b, ()):
                    R = batch_at[bb0][1]
                    kb = kb_seq[bb0]
                    ohb = ohpool.tile(
                        [128, OH_BATCH * KMAX * 128], bf16, name=f"oh{bb0}", tag="oh"
                    )
                    nc.vector.tensor_tensor(
                        out=ohb[:, 0:R * kb * 128]
                            .rearrange("p (r f c) -> p r f c", r=R, c=kb),
                        in0=iotafc_t[:, ioff[kb]:ioff[kb] + kb * 128]
                            .rearrange("p (o f c) -> p o f c", o=1, c=kb)
                            .to_broadcast([128, R, 128, kb]),
                        in1=lid_t[:, prefix[bb0]:prefix[bb0] + R * kb]
                            .rearrange("p (r o c) -> p r o c", r=R, o=1)
                            .to_broadcast([128, R, 128, kb]),
                        op=mybir.AluOpType.is_equal,
                    )
                    for i in range(R):
                        pending[bb0 + i] = ohb[:, i * kb * 128:(i + 1) * kb * 128]
                if b >= 1:
                    bb = b - 1
                    pending_ps[bb] = emit_matmuls(bb, xt_of.pop(bb), pending.pop(bb))
                if b >= 3:
                    pending_agg[b - 3] = emit_scale(b - 3, pending_ps.pop(b - 3))
                if b >= 5:
                    emit_tail(b - 5, pending_agg.pop(b - 5))
            last = nblocks - 1
            pending_ps[last] = emit_matmuls(last, xt_of.pop(last), pending.pop(last))
            for bb in sorted(pending_ps):
                pending_agg[bb] = emit_scale(bb, pending_ps.pop(bb))
            for bb in sorted(pending_agg):
                emit_tail(bb, pending_agg.pop(bb))

    nc.compile()
    return nc


def _get_module(kb_seq):
    key = tuple(kb_seq)
    if key not in _module_cache:
        _module_cache[key] = _build_module(key)
    return _module_cache[key]


def prepare_inputs(edge_data, dst, W, b):
    """Host-side sharding: route each edge to the core/block owning dst."""
    edge_data = np.asarray(edge_data, dtype=np.float32)
    dst = np.asarray(dst)
    W = np.asarray(W, dtype=np.float32)
    b = np.asarray(b, dtype=np.float32)
    E = dst.shape[0]

    # The device kernel divides by the raw count (no max(count,1) guard).
    # Give any zero-degree real node a phantom edge with zero features and a
    # unit count weight: sums stay exactly 0, so mean = 0/1 = 0, which
    # matches the reference's 0/max(0,1).
    node_cnt = np.bincount(dst, minlength=N_NODES)[:N_NODES]
    zeros = np.nonzero(node_cnt == 0)[0]
    n_real = E
    if len(zeros):
        dst = np.concatenate([dst, zeros.astype(dst.dtype)])
        E = dst.shape[0]

    blk = (dst.astype(np.int64)) >> 7                 # destination block id
    cnt = np.bincount(blk, minlength=TOTAL_BLOCKS)
    kb_all = np.maximum(1, -(-cnt // 128))            # chunks per block

    # Bin-pack: sort blocks by chunk count desc, deal round-robin to cores.
    # Every core then has the same chunk-count sequence kb_seq (per-position
    # max over cores = the first core's, since the deal preserves order).
    sortidx = np.argsort(-kb_all, kind="stable")
    core_of = np.empty(TOTAL_BLOCKS, np.int64)
    pos_of = np.empty(TOTAL_BLOCKS, np.int64)
    r = np.arange(TOTAL_BLOCKS)
    core_of[sortidx] = r % N_CORES
    pos_of[sortidx] = r // N_CORES
    kb_seq = kb_all[sortidx[0::N_CORES]]
    CH = int(kb_seq.sum())
    prefix = np.concatenate([[0], np.cumsum(kb_seq)]).astype(np.int64)

    starts = np.zeros(TOTAL_BLOCKS, np.int64)
    np.cumsum(cnt[:-1], out=starts[1:])
    order = np.argsort(blk, kind="stable")
    rank = np.empty(E, np.int64)
    rank[order] = np.arange(E, dtype=np.int64) - np.repeat(starts, cnt)

    # Flat slot in the per-core partition-major layout:
    #   (core*128 + partition) * CH + prefix[pos] + chunk
    slot = (
        (core_of[blk] * 128 + (rank & 127)) * CH
        + prefix[pos_of[blk]] + (rank >> 7)
    )

    FP8 = ml_dtypes.float8_e3m4
    X = np.zeros((N_CORES * 128 * CH, XCOL), FP8)
    X[slot[:n_real], 0:128] = edge_data.astype(FP8)
    X[slot[:n_real], 128] = FP8(1.0)
    if len(zeros):
        X[slot[n_real:], 128] = FP8(1.0)
    X = X.reshape(N_CORES, 128, CH * XCOL)

    lid_f = np.full(N_CORES * 128 * CH, -1.0, np.float32)
    lid_f[slot] = (dst & 127).astype(np.float32)
    lid_all = lid_f.reshape(N_CORES, 128, CH).astype(BF16)

    wt = np.ascontiguousarray(W.T).astype(BF16)
    bias = np.ascontiguousarray(b.reshape(128, 1))
    ident = np.eye(128, dtype=np.float32).astype(BF16)

    in_maps = [
        {
            "xe": np.ascontiguousarray(X[c]),
            "lid": np.ascontiguousarray(lid_all[c]),
            "wt": wt,
            "bias": bias,
            "ident": ident,
        }
        for c in range(N_CORES)
    ]
    return kb_seq, sortidx, in_maps


def run(edge_data, dst, W, b, trace=False, tmpdir=None):
    from concourse.bass_utils import run_bass_kernel_spmd

    kb_seq, sortidx, in_maps = prepare_inputs(edge_data, dst, W, b)
    nc = _get_module(kb_seq)
    res = run_bass_kernel_spmd(
        nc, in_maps, core_ids=list(range(N_CORES)), trace=trace, tmpdir=tmpdir,
    )
    full = np.empty((TOTAL_BLOCKS * 128, 128), np.float32)
    for c in range(N_CORES):
        oc = res.results[c]["out"].T.astype(np.float32)   # [12544, 128]
        blocks = sortidx[c::N_CORES]                      # block at position j
        for j, blkid in enumerate(blocks):
            full[blkid * 128:(blkid + 1) * 128] = oc[j * 128:(j + 1) * 128]
    full = full[:N_NODES]
    return np.ascontiguousarray(full, dtype=np.float32), res


def kernel(edge_data, dst, W, b):
    out, _ = run(edge_data, dst, W, b, trace=False)
    return out


# revision 52
# speedup vs baseline: 1.3763x; 1.0032x over previous
"""GCN edge-aggregation kernel for 8 Trainium2 NeuronCores.

Math (see nn_GCNEdge): h = relu((segment_sum(edge_data, dst) / max(count,1)) @ W.T + b)

Strategy
--------
Host-side (sharding/layout only — all arithmetic happens on device):
  * Nodes live in 784 blocks of 128; each edge is routed to the block owning
    its destination node (CSR-style destination binning).  Blocks are
    bin-packed onto the 8 cores: sorted by chunk count (ceil(edges/128)) and
    dealt round-robin, so every core sees the same per-position chunk-count
    sequence kb_seq and one SPMD program serves all cores, with only ~4% slot
    padding (vs ~13% for a uniform 18-chunk capacity).
  * Edge features ship as fp8 e3m4 (rel-err gate is 2e-2; e3m4 end-to-end is
    ~1.5e-2), with a constant-1 count column riding along for the degree counts.
  * The xe stream is partition-major: each SBUF partition's data for a run of
    blocks is one contiguous HBM range, so multi-block DMAs move ~30KB per
    partition per transfer (big descriptors -> full HBM bandwidth).

Device-side (per core, per 128-node block):
  * One-hot matrix of local node ids (DVE is_equal against an iota pattern).
    The one-hot is laid out [partition=edge, (node f, chunk c)] — f-major with
    the chunk axis innermost — so every DVE operand has a unit-stride last
    axis, which qualifies the op for the DVE 2x_1p fast path (2 elem/cycle).
    The PE matmul then reads each chunk's one-hot with a stride-K node axis.
  * PE matmul-accumulate onehot.T @ [x | 1 | 0] into PSUM -> per-node feature
    sums and counts,
  * mean = sums * reciprocal(count) (ACT copy with per-partition scale,
    casting to bf16),
  * PE transpose (bf16), then out = relu(W @ agg.T + b) via a bf16 matmul with
    the (pre-transposed) weight as the stationary operand; output stays
    transposed [out_feat, node] in bf16 and is un-transposed / upcast on host.

No collectives are needed: output shards are disjoint.
"""

import numpy as np
import ml_dtypes

BF16 = ml_dtypes.bfloat16

N_NODES = 100000
N_EDGES = 1600000
F = 128
N_CORES = 8
BLK = 128                       # nodes per block
BLOCKS_PER_CORE = 98
TOTAL_BLOCKS = N_CORES * BLOCKS_PER_CORE        # 784
NODES_PER_CORE = BLOCKS_PER_CORE * BLK          # 12544
XCOL = 129                      # 128 features + count col

_module_cache = {}


def _make_groups():
    """Positions per xe DMA transfer: tapered head so compute starts early,
    big groups in the middle for bandwidth, tapered tail so the final blocks'
    compute overlaps the last transfers."""
    return [2, 2, 3, 4, 5] + [6] * 13 + [2, 1, 1]


def _build_module(kb_seq):
    import concourse.mybir as mybir
    import concourse.tile as tile
    from concourse import bacc

    f32 = mybir.dt.float32
    bf16 = mybir.dt.bfloat16
    fp8 = mybir.dt.float8e3

    kb_seq = list(kb_seq)
    nblocks = len(kb_seq)
    CH = sum(kb_seq)                       # total chunks per core
    prefix = np.concatenate([[0], np.cumsum(kb_seq)]).astype(int)
    kdistinct = sorted(set(kb_seq))
    ioff = {}
    o = 0
    for k in kdistinct:
        ioff[k] = o
        o += k * 128
    IOTA_COLS = o
    KMAX = max(kb_seq)

    groups = _make_groups()
    assert sum(groups) == nblocks
    gstart = np.concatenate([[0], np.cumsum(groups)]).astype(int)
    GMAXCH = max(
        prefix[gstart[gi + 1]] - prefix[gstart[gi]] for gi in range(len(groups))
    )

    # Greedy byte-balance the xe groups over the two hardware DMA queues
    # (SP, ACT); ACT is pre-loaded with the lid constant and the out writes.
    gbytes = [
        float(prefix[gstart[gi + 1]] - prefix[gstart[gi]]) for gi in range(len(groups))
    ]
    act_extra = 2 * (nblocks * 128) / XCOL  # out writes (2B/elem), in xe-chunk units
    sp_extra = 2 * CH / XCOL                # lid rides the SP queue
    load = {0: sp_extra, 1: act_extra}
    gqueue = []
    for w in gbytes:
        qsel = 0 if load[0] <= load[1] else 1
        gqueue.append(qsel)
        load[qsel] += w

    nc = bacc.Bacc("TRN2", target_bir_lowering=False, debug=False)
    xe = nc.dram_tensor("xe", [128, CH * XCOL], fp8, kind="ExternalInput")
    lid = nc.dram_tensor("lid", [128, CH], bf16, kind="ExternalInput")
    wt = nc.dram_tensor("wt", [128, 128], bf16, kind="ExternalInput")
    bias = nc.dram_tensor("bias", [128, 1], f32, kind="ExternalInput")
    ident = nc.dram_tensor("ident", [128, 128], bf16, kind="ExternalInput")
    out = nc.dram_tensor("out", [128, nblocks * 128], bf16, kind="ExternalOutput")

    xe_ap = xe.ap()
    out_ap = out.ap()

    with tile.TileContext(nc) as tc:
        with (
            tc.tile_pool(name="const", bufs=1) as cpool,
            tc.tile_pool(name="xp", bufs=5) as xpool,
            tc.tile_pool(name="ohp", bufs=8) as ohpool,
            tc.tile_pool(name="ep", bufs=3) as epool,
            tc.tile_pool(name="psS", bufs=4, space="PSUM") as psS,
            tc.tile_pool(name="psT", bufs=2, space="PSUM") as psT,
            tc.tile_pool(name="psO", bufs=2, space="PSUM") as psO,
        ):
            # Constants ride the ACT engine's hardware DMA queue so the SP
            # queue starts the bulk xe stream immediately; the one-hot
            # prerequisite (lid) loads first.  The iota compare patterns
            # (iotafc[p, ioff[k] + f*k + c] = f, one per distinct kb; values
            # 0..127 are exact in bf16) are generated on the idle GPSIMD
            # engine instead of being shipped over HBM — the pattern used by
            # the first blocks is generated first.
            lid_t = cpool.tile([128, CH], bf16)
            nc.sync.dma_start(lid_t[:], lid.ap()[:])
            wt_t = cpool.tile([128, 128], bf16)
            nc.scalar.dma_start(wt_t[:], wt.ap()[:])
            bias_t = cpool.tile([128, 1], f32)
            nc.scalar.dma_start(bias_t[:], bias.ap()[:])
            id_t = cpool.tile([128, 128], bf16)
            nc.scalar.dma_start(id_t[:], ident.ap()[:])
            iotafc_t = cpool.tile([128, IOTA_COLS], bf16)
            for k in sorted(kdistinct, reverse=True):
                nc.gpsimd.iota(
                    iotafc_t[:, ioff[k]:ioff[k] + k * 128],
                    [[1, 128], [0, k]],
                    channel_multiplier=0,
                    allow_small_or_imprecise_dtypes=True,
                )

            group_pT = {}

            def emit_matmuls(b, xt, oh):
                kb = kb_seq[b]
                ps = psS.tile([128, XCOL], f32, name=f"ps{b}", tag="ps")
                ohv = oh[:, 0:kb * 128].rearrange("p (f c) -> p c f", c=kb)
                for c in range(kb):
                    nc.tensor.matmul(
                        ps[:],
                        lhsT=ohv[:, c, :],
                        rhs=xt[:, c * XCOL:(c + 1) * XCOL],
                        start=(c == 0),
                        stop=(c == kb - 1),
                    )
                return ps

            def emit_scale(b, ps):
                # counts live in ps[:,128].  No max(count,1) guard: the host
                # guarantees every real node has count > 0 (injecting
                # zero-feature phantom edges if needed); padding nodes divide
                # by zero -> NaN columns that the host slices off.
                rec = epool.tile([128, 1], f32, name=f"rec{b}", tag="rec")
                nc.vector.reciprocal(rec[:], ps[:, 128:129])
                agg = epool.tile([128, 128], bf16, name=f"agg{b}", tag="agg", bufs=5)
                nc.scalar.activation(
                    agg[:], ps[:, 0:128],
                    mybir.ActivationFunctionType.Copy, scale=rec[:, 0:1],
                )
                return agg

            def emit_tail(b, agg):
                j = b % 4
                if j == 0:
                    group_pT["t"] = psT.tile([128, 512], bf16, name=f"pT{b}", tag="pT")
                pT = group_pT["t"]
                nc.tensor.transpose(pT[:, j * 128:(j + 1) * 128], agg[:], id_t[:])
                if j == 3 or b == nblocks - 1:
                    g0 = (b // 4) * 4
                    gw = (b + 1 - g0) * 128
                    aggT = epool.tile([128, 512], bf16, name=f"aggT{b}", tag="aggT", bufs=3)
                    nc.scalar.copy(aggT[:, 0:gw], pT[:, 0:gw])
                    pO = psO.tile([128, 512], f32, name=f"pO{b}", tag="pO")
                    nc.tensor.matmul(
                        pO[:, 0:gw], lhsT=wt_t[:], rhs=aggT[:, 0:gw],
                        start=True, stop=True,
                    )
                    ot = epool.tile([128, 512], bf16, name=f"ot{b}", tag="ot", bufs=4)
                    nc.scalar.activation(
                        ot[:, 0:gw], pO[:, 0:gw],
                        mybir.ActivationFunctionType.Relu,
                        bias=bias_t[:, 0:1], scale=1.0,
                    )
                    nc.scalar.dma_start(out_ap[:, g0 * 128:(b + 1) * 128], ot[:, 0:gw])

            # One is_equal builds the one-hots for a whole run of same-kb
            # blocks (up to OH_BATCH), amortizing the per-op fixed cost on
            # the DVE (the busiest engine).
            OH_BATCH = 1
            batches = []              # (b0, R)
            b0 = 0
            while b0 < nblocks:
                R = 1
                while (
                    R < OH_BATCH
                    and b0 + R < nblocks
                    and kb_seq[b0 + R] == kb_seq[b0]
                ):
                    R += 1
                batches.append((b0, R))
                b0 += R
            batch_at = {b0: (b0, R) for b0, R in batches}
            OH_LEAD = 3
            emit_at = {}
            for b0, R in batches:
                emit_at.setdefault(max(0, b0 - OH_LEAD), []).append(b0)

            # Software-pipelined emission. Every engine queue is strict
            # in-order, so an op gated on *fresh* upstream state stalls the
            # whole queue behind it. Stagger each stage so, by the time a
            # queue reaches an op, its dependencies are blocks old:
            #   iter b:  DMA xe group | one-hot batch | PE matmuls(b-1)
            #            | reciprocal+scale of (b-3) | transpose/output of (b-5)
            xt_of = {}
            gi = 0
            pending = {}
            pending_ps = {}
            pending_agg = {}
            for b in range(nblocks):
                if gi < len(groups) and b == gstart[gi]:
                    c0, c1 = prefix[gstart[gi]], prefix[gstart[gi + 1]]
                    xg = xpool.tile([128, GMAXCH * XCOL], fp8, name=f"xg{gi}", tag="xg")
                    # Both hardware DMA queues (SP and ACT) pull HBM
                    # concurrently, byte-balanced.
                    dma_eng = nc.sync if gqueue[gi] == 0 else nc.scalar
                    dma_eng.dma_start(
                        xg[:, 0:(c1 - c0) * XCOL],
                        xe_ap[:, c0 * XCOL:c1 * XCOL],
                    )
                    for bb in range(gstart[gi], gstart[gi + 1]):
                        off = (prefix[bb] - c0) * XCOL
                        xt_of[bb] = xg[:, off:off + kb_seq[bb] * XCOL]
                    gi += 1
                # Emit one-hot batches a few iterations ahead of their first
                # consumer so PE never waits on a freshly-issued DVE op.
                for bb0 in emit_at.get(b, ()):
                    R = batch_at[bb0][1]
                    kb = kb_seq[bb0]
                    ohb = ohpool.tile(
                        [128, OH_BATCH * KMAX * 128], bf16, name=f"oh{bb0}", tag="oh"
                    )
                    nc.vector.tensor_tensor(
                        out=ohb[:, 0:R * kb * 128]
                            .rearrange("p (r f c) -> p r f c", r=R, c=kb),
                        in0=iotafc_t[:, ioff[kb]:ioff[kb] + kb * 128]
                            .rearrange("p (o f c) -> p o f c", o=1, c=kb)
                            .to_broadcast([128, R, 128, kb]),
                        in1=lid_t[:, prefix[bb0]:prefix[bb0] + R * kb]
                            .rearrange("p (r o c) -> p r o c", r=R, o=1)
                            .to_broadcast([128, R, 128, kb]),
                        op=mybir.AluOpType.is_equal,
                    )
                    for i in range(R):
                        pending[bb0 + i] = ohb[:, i * kb * 128:(i + 1) * kb * 128]
                if b >= 1:
                    bb = b - 1
                    pending_ps[bb] = emit_matmuls(bb, xt_of.pop(bb), pending.pop(bb))
                if b >= 2:
                    pending_agg[b - 2] = emit_scale(b - 2, pending_ps.pop(b - 2))
                if b >= 4:
                    emit_tail(b - 4, pending_agg.pop(b - 4))
            last = nblocks - 1
            pending_ps[last] = emit_matmuls(last, xt_of.pop(last), pending.pop(last))
            for bb in sorted(pending_ps):
                pending_agg[bb] = emit_scale(bb, pending_ps.pop(bb))
            for bb in sorted(pending_agg):
                emit_tail(bb, pending_agg.pop(bb))

    nc.compile()
    return nc


def _get_module(kb_seq):
    key = tuple(kb_seq)
    if key not in _module_cache:
        _module_cache[key] = _build_module(key)
    return _module_cache[key]


def prepare_inputs(edge_data, dst, W, b):
    """Host-side sharding: route each edge to the core/block owning dst."""
    edge_data = np.asarray(edge_data, dtype=np.float32)
    dst = np.asarray(dst)
    W = np.asarray(W, dtype=np.float32)
    b = np.asarray(b, dtype=np.float32)
    E = dst.shape[0]

    # The device kernel divides by the raw count (no max(count,1) guard).
    # Give any zero-degree real node a phantom edge with zero features and a
    # unit count weight: sums stay exactly 0, so mean = 0/1 = 0, which
    # matches the reference's 0/max(0,1).
    node_cnt = np.bincount(dst, minlength=N_NODES)[:N_NODES]
    zeros = np.nonzero(node_cnt == 0)[0]
    n_real = E
    if len(zeros):
        dst = np.concatenate([dst, zeros.astype(dst.dtype)])
        E = dst.shape[0]

    blk = (dst.astype(np.int64)) >> 7                 # destination block id
    cnt = np.bincount(blk, minlength=TOTAL_BLOCKS)
    kb_all = np.maximum(1, -(-cnt // 128))            # chunks per block

    # Bin-pack: sort blocks by chunk count desc, deal round-robin to cores.
    # Every core then has the same chunk-count sequence kb_seq (per-position
    # max over cores = the first core's, since the deal preserves order).
    sortidx = np.argsort(-kb_all, kind="stable")
    core_of = np.empty(TOTAL_BLOCKS, np.int64)
    pos_of = np.empty(TOTAL_BLOCKS, np.int64)
    r = np.arange(TOTAL_BLOCKS)
    core_of[sortidx] = r % N_CORES
    pos_of[sortidx] = r // N_CORES
    kb_seq = kb_all[sortidx[0::N_CORES]]
    CH = int(kb_seq.sum())
    prefix = np.concatenate([[0], np.cumsum(kb_seq)]).astype(np.int64)

    starts = np.zeros(TOTAL_BLOCKS, np.int64)
    np.cumsum(cnt[:-1], out=starts[1:])
    order = np.argsort(blk, kind="stable")
    rank = np.empty(E, np.int64)
    rank[order] = np.arange(E, dtype=np.int64) - np.repeat(starts, cnt)

    # Flat slot in the per-core partition-major layout:
    #   (core*128 + partition) * CH + prefix[pos] + chunk
    slot = (
        (core_of[blk] * 128 + (rank & 127)) * CH
        + prefix[pos_of[blk]] + (rank >> 7)
    )

    FP8 = ml_dtypes.float8_e3m4
    X = np.zeros((N_CORES * 128 * CH, XCOL), FP8)
    X[slot[:n_real], 0:128] = edge_data.astype(FP8)
    X[slot[:n_real], 128] = FP8(1.0)
    if len(zeros):
        X[slot[n_real:], 128] = FP8(1.0)
    X = X.reshape(N_CORES, 128, CH * XCOL)

    lid_f = np.full(N_CORES * 128 * CH, -1.0, np.float32)
    lid_f[slot] = (dst & 127).astype(np.float32)
    lid_all = lid_f.reshape(N_CORES, 128, CH).astype(BF16)

    wt = np.ascontiguousarray(W.T).astype(BF16)
    bias = np.ascontiguousarray(b.reshape(128, 1))
    ident = np.eye(128, dtype=np.float32).astype(BF16)

    in_maps = [
        {
            "xe": np.ascontiguousarray(X[c]),
            "lid": np.ascontiguousarray(lid_all[c]),
            "wt": wt,
            "bias": bias,
            "ident": ident,
        }
        for c in range(N_CORES)
    ]
    return kb_seq, sortidx, in_maps


def run(edge_data, dst, W, b, trace=False, tmpdir=None):
    from concourse.bass_utils import run_bass_kernel_spmd

    kb_seq, sortidx, in_maps = prepare_inputs(edge_data, dst, W, b)
    nc = _get_module(kb_seq)
    res = run_bass_kernel_spmd(
        nc, in_maps, core_ids=list(range(N_CORES)), trace=trace, tmpdir=tmpdir,
    )
    full = np.empty((TOTAL_BLOCKS * 128, 128), np.float32)
    for c in range(N_CORES):
        oc = res.results[c]["out"].T.astype(np.float32)   # [12544, 128]
        blocks = sortidx[c::N_CORES]                      # block at position j
        for j, blkid in enumerate(blocks):
            full[blkid * 128:(blkid + 1) * 128] = oc[j * 128:(j + 1) * 128]
    full = full[:N_NODES]
    return np.ascontiguousarray(full, dtype=np.float32), res


def kernel(edge_data, dst, W, b):
    out, _ = run(edge_data, dst, W, b, trace=False)
    return out
